# revision 32
# baseline (speedup 1.0000x reference)
"""CostVolume Trainium2 kernel (Bass/Tile), SPMD over 8 NeuronCores.

Problem (hardcoded shapes):
  x, y: [2, 64, 96, 320] f32.  GROUP=8, cpg=8, MAXDISP=48, D=49.
  out:  [2, 8, 49, 96, 320] f32
  out[b,g,k,h,j] = sum_c | xn[b,g,c,h,j] - yn_pad[b,g,c,h,j+48-k] |
  where xn/yn are channel-group L2-normalized (cpg=8) and yn_pad has 48 zero
  columns prepended along w.

Sharding: 16 independent (b, g) pairs -> 2 per core.

Per-core design (v4):
  SBUF partition p = c*16 + hh (c in 0..8, hh in 0..16); free = (ht, w),
  h = 16*ht + hh.  Disparities processed in TRIADS of 3 consecutive k
  (ascending within the triad via a negative-stride overlapping-window AP on
  the padded y) and a stride-0 broadcast of xn over the triad dim.

  Path variants per triad (mix tuned to balance engines):
   'm': DVE min (fp16 2x); |a-b| = a+b-2min via merged matmuls: one id96
        U-broadcast matmul + one id96 V-window matmul + 6 s6n(-2) matmuls,
        all writing the whole [H, nwin, w0] PSUM region at once.
   'p': same as 'm' but the min runs on the Pool (gpsimd) engine, and so
        does the PSUM->SBUF copy.
   'q': DVE subtract -> ACT Abs emitting fp8e4 -> 3 PE DoubleRow matmuls
        (K=256, 0.5 cyc/row) with selector weights summing channels.
  Each triad accumulates into a [96, 3, 512] PSUM tile; the engine copies the
  valid region to an fp16 [H, 3, W] tile whose pad strip [0:js) is filled
  from the precomputed sum|xn| strip (a3), so ONE DMA per triad (riding the
  ACT hwdge queue, deps already retired) stores all three disparities at
  full width.  Host converts fp16 -> f32.

  bg1's normalization is emitted one step per triad between bg0's triads so
  its ACT/PE work never head-of-line blocks the triad pipeline.
"""

import numpy as np

B, C, H, W = 2, 64, 96, 320
GROUP = 8
CPG = C // GROUP          # 8
MAXDISP = 48
D = MAXDISP + 1           # 49
NCORES = 8
BG_TOTAL = B * GROUP      # 16
BG_PER_CORE = BG_TOTAL // NCORES  # 2
HH = 16
HT = H // HH              # 6
PW = 376                  # padded y width (48 zeros + 320 + 8 zero tail)
NG = 17                   # triads per bg (k=48..1), plus single k=0

# path per group index g=0..16 (g<16: triad kmax=48-3g; g=16: single k=0)
import os as _os
PATHS0 = list(_os.environ.get("CV_PATHS0", "mmqmmmmmqmmmqmmmm"))
PATHS1 = list(_os.environ.get("CV_PATHS1", "mmqmmmmmqmmmqmmmm"))
_YQ = int(_os.environ.get("CV_YQ", "0"))      # 1: bg0 y-loads on Pool SWDGE
_MULP = int(_os.environ.get("CV_MULP", "0"))  # Pool cannot run TensorTensor (ISA)
_PUMP = int(_os.environ.get("CV_PUMP", "1"))  # norm1 steps pumped per triad
_DPOOL = int(_os.environ.get("CV_DPOOL", "6"))
_CHUNK = int(_os.environ.get("CV_CHUNK", "3"))  # ht rows per norm chunk
_QCH = int(_os.environ.get("CV_QCH", "3"))      # abs chunks per q triad
_QF = int(_os.environ.get("CV_QF", "0"))        # 1: copy before abs
_OPOOL = int(_os.environ.get("CV_OPOOL", "8"))

_PROG = None


def _constants():
    import ml_dtypes

    # s0b[r, p] = 1 iff r % 16 == p % 16: sums the 8 channels and replicates
    # the result across all 128 partitions in one matmul (no broadcast pass)
    s0 = np.zeros((128, 128), np.float32)
    for p in range(128):
        for c in range(CPG):
            s0[c * HH + p % HH, p] = 1.0
    # s66[0:6] = +1 selectors (U/V/a3 prep), s66[6:12] = -2 selectors (min)
    s66 = np.zeros((2 * HT, 128, H), np.float16)
    for t in range(HT):
        for p in range(128):
            s66[t, p, HH * t + p % HH] = 1.0
            s66[HT + t, p, HH * t + p % HH] = -2.0
    s6dr = np.zeros((HT // 2, 128, 2, H), np.float32)
    for tp in range(HT // 2):
        for p in range(128):
            for i in range(2):
                s6dr[tp, p, i, 32 * tp + 16 * i + p % HH] = 1.0
    id96 = np.eye(H, dtype=np.float16)
    return {
        "s0f32": s0.astype(np.float16),
        "s66f16": s66,
        "s6dr8": s6dr.astype(ml_dtypes.float8_e4m3),
        "id96": id96,
    }


def _build():
    global _PROG
    if _PROG is not None:
        return _PROG

    import bass_rust
    import concourse.bacc as bacc
    import concourse.mybir as mybir
    import concourse.tile as tile

    VP = bass_rust.VecI64Pair
    f32 = mybir.dt.float32
    f32r = mybir.dt.float32r
    f16 = mybir.dt.float16
    f8e4 = mybir.dt.float8e4
    u16 = mybir.dt.uint16
    AF = mybir.ActivationFunctionType
    ALU = mybir.AluOpType
    PM = mybir.MatmulPerfMode

    nc = bacc.Bacc("TRN2", target_bir_lowering=False, debug=False)

    xin = nc.dram_tensor("x", [BG_PER_CORE, 128, HT, W], f32, kind="ExternalInput")
    yin = nc.dram_tensor("y", [BG_PER_CORE, 128, HT, W], f32, kind="ExternalInput")
    s0_d = nc.dram_tensor("s0f32", [128, 128], f16, kind="ExternalInput")
    s66_d = nc.dram_tensor("s66f16", [2 * HT, 128, H], f16, kind="ExternalInput")
    s6dr_d = nc.dram_tensor("s6dr8", [HT // 2, 128, 2, H], f8e4, kind="ExternalInput")
    id96_d = nc.dram_tensor("id96", [H, H], f16, kind="ExternalInput")
    out_d = nc.dram_tensor("out", [BG_PER_CORE, D, H, W], f16, kind="ExternalOutput")

    x_v = xin.ap()
    y_v = yin.ap()
    out_v = out_d.ap()

    def windows_desc(base_ap, last_start, nwin, width):
        """Overlapping windows on the last dim with DESCENDING start:
        window i reads cols [last_start - i, last_start - i + width)."""
        ap = base_ap[..., last_start:last_start + width].copy()
        dims = [tuple(v) for v in ap.ap]
        ap.ap = VP(dims[:-1] + [(-1, nwin), (1, width)])
        return ap

    with tile.TileContext(nc) as tc:
        with (
            tc.tile_pool(name="const", bufs=1) as constp,
            tc.tile_pool(name="stage", bufs=2) as stagep,
            tc.tile_pool(name="norm", bufs=2) as normp,
            tc.tile_pool(name="keep", bufs=1) as keepp,
            tc.tile_pool(name="nps", bufs=2, space="PSUM") as npsp,
            tc.tile_pool(name="mmps", bufs=2, space="PSUM") as mmpsp,
            tc.tile_pool(name="dpool", bufs=_DPOOL) as dpool,
            tc.tile_pool(name="ppool", bufs=3) as ppool,
            tc.tile_pool(name="qpool", bufs=3) as qpool,
            tc.tile_pool(name="opool", bufs=_OPOOL) as opool,
        ):
            # constants ride the idle Pool SWDGE queue, batched into 5 DMAs so
            # they don't occupy the Pool engine for long
            s0_t = constp.tile([128, 128], f16, tag="s0")
            id96 = constp.tile([H, H], f16, tag="id96")
            s66_t = constp.tile([128, 2 * HT, H], f16, tag="s66")
            s6dr_t = constp.tile([128, HT // 2, 2, H], f8e4, tag="s6dr")
            nc.gpsimd.dma_start(s0_t[:], s0_d.ap())
            consts_emitted = [False]

            def emit_weight_consts():
                if consts_emitted[0]:
                    return
                consts_emitted[0] = True
                nc.gpsimd.dma_start(id96[:], id96_d.ap())
                nc.gpsimd.dma_start(
                    s66_t[:], s66_d.ap().rearrange("t p h -> p t h"))
                nc.gpsimd.dma_start(
                    s6dr_t[:], s6dr_d.ap().rearrange("q p i h -> p q i h"))
            s6 = [s66_t[:, t] for t in range(HT)]
            s6n = [s66_t[:, HT + t] for t in range(HT)]
            s6dr = [s6dr_t[:, tp] for tp in range(HT // 2)]

            # persistent per-bg tiles
            xn, ynp, u1, vp, a3 = [], [], [], [], []
            for bg in range(BG_PER_CORE):
                xn.append(keepp.tile([128, HT, W], f16, tag=f"xn{bg}", name=f"xn{bg}"))
                ynp.append(keepp.tile([128, HT, PW], f16, tag=f"ynp{bg}", name=f"ynp{bg}"))
                u1.append(keepp.tile([H, W], f16, tag=f"u1{bg}", name=f"u1{bg}"))
                vp.append(keepp.tile([H, PW], f16, tag=f"vp{bg}", name=f"vp{bg}"))
                a3.append(keepp.tile([H, 3, 47], f16, tag=f"a3{bg}", name=f"a3{bg}"))

            # ---------------- normalization + U/V prep ----------------
            def emit_norm(bg):
                """Generator: emits normalization for one bg in ~10 steps."""
                nc.gpsimd.memset(ynp[bg][:, :, 0:MAXDISP], 0.0)
                nc.gpsimd.memset(ynp[bg][:, :, MAXDISP + W:], 0.0)
                raws = []
                sqs = []
                rss = []
                for is_y in (0, 1):
                    nm = "y" if is_y else "x"
                    raws.append(stagep.tile(
                        [128, HT, W], f32 if (bg == 0 and not is_y) else f16,
                        tag="raw", name=f"raw{nm}"))
                    sqs.append(stagep.tile(
                        [128, HT, W], f16, tag="sq", name=f"sq{nm}"))
                    rss.append(normp.tile(
                        [128, HT, W], f16, tag="rs", name=f"rs{nm}"))
                yield
                # pipelined per chunk: load -> square -> per-t ssum/rsqrt/mul
                for c0 in range(0, HT, _CHUNK):
                    sl = slice(c0, c0 + _CHUNK)
                    for is_y in (0, 1):
                        src_v = y_v if is_y else x_v
                        raw, sq, rs = raws[is_y], sqs[is_y], rss[is_y]
                        # f32 -> f16 cast happens inside the SWDGE DMA, so
                        # the norm multiply runs at DVE 2x on all-f16 operands.
                        # bg0's x rides SP uncast: Pool's serial desc-gen would
                        # otherwise pace the whole prologue.
                        if bg == 0 and not is_y:
                            nc.sync.dma_start(raw[:, sl], src_v[bg][:, sl])
                        else:
                            nc.gpsimd.dma_start(raw[:, sl], src_v[bg][:, sl])
                        nc.scalar.activation(
                            sq[:, sl].rearrange("p a b -> p (a b)"),
                            raw[:, sl].rearrange("p a b -> p (a b)"),
                            AF.Square,
                        )
                        for t in range(c0, c0 + _CHUNK):
                            ssum = npsp.tile([128, 512], f32, tag="ssum")
                            nc.tensor.matmul(
                                ssum[:, 0:W], s0_t[:], sq[:, t, :],
                                start=True, stop=True,
                            )
                            nc.scalar.activation(
                                rs[:, t, :], ssum[:, 0:W], AF.Abs_reciprocal_sqrt
                            )
                        if is_y:
                            nc.vector.tensor_mul(
                                ynp[bg][:, sl, MAXDISP:MAXDISP + W],
                                raw[:, sl], rs[:, sl],
                            )
                        else:
                            nc.vector.tensor_mul(
                                xn[bg][:, sl], raw[:, sl], rs[:, sl]
                            )
                        yield
                    if bg == 0 and c0 == 0:
                        emit_weight_consts()
                # U = sum_c xn -> [96, W]; V = sum_c ynp -> [96, PW]
                ups = npsp.tile([128, 512], f32, tag="ssum", name="ups")
                for t in range(HT):
                    nc.tensor.matmul(
                        ups[0:H, 0:W], s6[t], xn[bg][:, t, :],
                        start=(t == 0), stop=(t == HT - 1),
                    )
                nc.scalar.activation(u1[bg][:], ups[0:H, 0:W], AF.Copy)
                yield
                vps = npsp.tile([128, 512], f32, tag="ssum", name="vps")
                for t in range(HT):
                    nc.tensor.matmul(
                        vps[0:H, 0:PW], s6[t], ynp[bg][:, t, :],
                        start=(t == 0), stop=(t == HT - 1),
                    )
                nc.scalar.activation(vp[bg][:], vps[0:H, 0:PW], AF.Copy)
                yield
                # A3[h, kk, j] = sum_c |xn_c[h, j]| for j < 46: the pad-strip
                # output (identical for every k in a triad), stored 3-wide.
                ax = normp.tile([128, HT, 48], f16, tag="ax")
                nc.scalar.activation(
                    ax[:, :, 0:47], xn[bg][:, :, 0:47], AF.Abs,
                )
                aps = npsp.tile([128, 512], f32, tag="ssum", name="aps")
                for t in range(HT):
                    nc.tensor.matmul(
                        aps[0:H, 0:47], s6[t], ax[:, t, 0:47],
                        start=(t == 0), stop=(t == HT - 1),
                    )
                nc.scalar.activation(
                    a3[bg][:],
                    aps[0:H, 0:47].unsqueeze(1).broadcast_to([H, 3, 47]),
                    AF.Copy,
                )
                yield

            for _ in emit_norm(0):
                pass
            norm1 = emit_norm(1)

            # ---------------- main loop: triads ----------------
            # 1-triad software-pipeline skew: triad g's PSUM->SBUF copy and
            # store are emitted AFTER triad g+1's elementwise/abs work, so the
            # in-order ACT queue runs the next abs before stalling on the
            # copy's PSUM dependency.
            pending = None

            def flush_pending():
                nonlocal pending
                if pending is None:
                    return
                p_bg, p_ps, p_ob, p_nwin, p_kmax, p_js, p_path = pending
                w0 = W - p_js
                if p_path == "t":
                    nc.vector.tensor_scalar(
                        p_ob[:, 0:p_nwin, p_js:], p_ps[:, 0:p_nwin, 0:w0],
                        0.0, None, op0=ALU.add,
                    )
                else:
                    nc.scalar.activation(
                        p_ob[:, 0:p_nwin, p_js:], p_ps[:, 0:p_nwin, 0:w0],
                        AF.Copy,
                    )
                klo = p_kmax - p_nwin + 1
                # stores ride the ACT hwdge queue: their data deps retire just
                # before them there, so the SEQ isn't parked on long semaphore
                # waits (a single SP queue serialized the whole kernel on
                # those waits).  The pad strip [0:js) goes straight from a3
                # via a second tiny DMA -- no engine op at all.
                nc.scalar.dma_start(
                    out_v[p_bg, klo:p_kmax + 1, :, p_js:].rearrange(
                        "k h w -> h k w"),
                    p_ob[:, 0:p_nwin, p_js:],
                )
                if p_js > 0:
                    nc.sync.dma_start(
                        out_v[p_bg, klo:p_kmax + 1, :, 0:p_js].rearrange(
                            "k h w -> h k w"),
                        a3[p_bg][:, 0:p_nwin, 0:p_js],
                    )
                pending = None

            for bg in range(BG_PER_CORE):
                for g in range(NG):
                    path = (PATHS0 if bg == 0 else PATHS1)[g]
                    if g < NG - 1:
                        kmax = MAXDISP - 3 * g       # triad k = kmax-2..kmax
                        nwin = 3
                        base = MAXDISP - kmax        # 48 - k for k = kmax
                    else:
                        kmax = 0
                        nwin = 1
                        base = MAXDISP
                    js = kmax - nwin + 1             # valid window: j >= js
                    w0 = W - js
                    # window kk corresponds to k = js + kk (ascending):
                    # ynp col = (j - js) + base + js + (nwin-1) - kk
                    ywin = windows_desc(
                        ynp[bg][:], base + js + (nwin - 1), nwin, w0)
                    xin_ap = (
                        xn[bg][:, :, js:].unsqueeze(2)
                        .broadcast_to([128, HT, nwin, w0])
                    )

                    ps = mmpsp.tile([H, 3, 512], f32, tag="ps")
                    ob = opool.tile([H, 3, W], f16, tag="ob")
                    if path in ("m", "p"):
                        tail = bg == BG_PER_CORE - 1 and g >= NG - 2
                        m3 = dpool.tile([128, HT, 3, W], f16, tag="d3", name="m3")
                        if tail:
                            # drain the pipeline: chunked mins let PE start
                            # before the whole min lands
                            for c0 in range(0, HT, 2):
                                nc.vector.tensor_tensor(
                                    m3[:, c0:c0 + 2, 0:nwin, 0:w0],
                                    xn[bg][:, c0:c0 + 2, js:].unsqueeze(2)
                                    .broadcast_to([128, 2, nwin, w0]),
                                    windows_desc(
                                        ynp[bg][:, c0:c0 + 2],
                                        base + js + (nwin - 1), nwin, w0),
                                    ALU.min,
                                )
                        else:
                            nc.vector.tensor_tensor(
                                m3[:, :, 0:nwin, 0:w0], xin_ap, ywin, ALU.min
                            )
                        flush_pending()
                        # per k: U + V_k + sum_c min (multi-bank matmul
                        # outputs fail the real ISA's s3d3 check, so one
                        # bank per matmul)
                        for kk in range(nwin):
                            vs = base + js + (nwin - 1) - kk
                            nc.tensor.matmul(
                                ps[:, kk, 0:w0], id96[:], u1[bg][:, js:],
                                start=True, stop=False,
                            )
                            nc.tensor.matmul(
                                ps[:, kk, 0:w0], id96[:],
                                vp[bg][:, vs:vs + w0],
                                start=False, stop=False,
                            )
                        for t in range(HT):
                            for kk in range(nwin):
                                nc.tensor.matmul(
                                    ps[:, kk, 0:w0], s6n[t],
                                    m3[:, t, kk, 0:w0],
                                    start=False, stop=(t == HT - 1),
                                )
                    else:  # 'q'
                        d3 = dpool.tile([128, HT, 3, W], f16, tag="d3", name="d3")
                        nc.vector.tensor_tensor(
                            d3[:, :, 0:nwin, 0:w0], xin_ap, ywin, ALU.subtract
                        )
                        q3 = qpool.tile([128, HT, 3, W], f8e4, tag="q3", name="q3")
                        # abs in per-2ht chunks so the PSUM-freeing copy of the
                        # previous group isn't head-of-line blocked behind one
                        # long ACT op
                        qch = HT // _QCH
                        if _QF:
                            flush_pending()
                        nc.scalar.activation(
                            q3[:, 0:qch, 0:nwin, 0:w0],
                            d3[:, 0:qch, 0:nwin, 0:w0],
                            AF.Abs,
                        )
                        if not _QF:
                            flush_pending()
                        for ci in range(1, _QCH):
                            nc.scalar.activation(
                                q3[:, ci * qch:(ci + 1) * qch, 0:nwin, 0:w0],
                                d3[:, ci * qch:(ci + 1) * qch, 0:nwin, 0:w0],
                                AF.Abs,
                            )
                        for kk in range(nwin):
                            for tp in range(HT // 2):
                                nc.tensor.matmul(
                                    ps[:, kk, 0:w0], s6dr[tp],
                                    q3[:, 2 * tp:2 * tp + 2, kk, 0:w0],
                                    start=(tp == 0), stop=(tp == HT // 2 - 1),
                                    perf_mode=PM.DoubleRow,
                                )

                    pending = (bg, ps, ob, nwin, kmax, js, path)
                    if bg == 0:
                        for _ in range(_PUMP):
                            next(norm1, None)
                if bg == 0:
                    for _ in norm1:
                        pass
            flush_pending()

    nc.compile()
    _PROG = nc
    return nc


def run(x, y, trace=False, trace_kwargs=None):
    """x, y: full [2, 64, 96, 320] f32. Returns (out [2,8,49,96,320] f32, res)."""
    from concourse import bass_utils

    nc = _build()
    consts = _constants()

    def _prep(a):
        a = np.asarray(a, np.float32).reshape(BG_TOTAL, CPG, HT, HH, W)
        return np.ascontiguousarray(a.transpose(0, 1, 3, 2, 4)).reshape(
            BG_TOTAL, 128, HT, W
        )

    xr = _prep(x)
    yr = _prep(y)
    in_maps = []
    for i in range(NCORES):
        sl = slice(i * BG_PER_CORE, (i + 1) * BG_PER_CORE)
        in_maps.append(
            {
                "x": np.ascontiguousarray(xr[sl]),
                "y": np.ascontiguousarray(yr[sl]),
                **consts,
            }
        )
    res = bass_utils.run_bass_kernel_spmd(
        nc,
        in_maps,
        core_ids=list(range(NCORES)),
        trace=trace,
        **(trace_kwargs or {}),
    )
    full = np.concatenate(
        [np.asarray(r["out"], np.float32) for r in res.results], axis=0
    )
    out = full.reshape(B, GROUP, D, H, W)
    return out, res


def kernel(x, y):
    out, _ = run(x, y, trace=False)
    return out


# revision 33
# speedup vs baseline: 1.0014x; 1.0014x over previous
"""CostVolume Trainium2 kernel (Bass/Tile), SPMD over 8 NeuronCores.

Problem (hardcoded shapes):
  x, y: [2, 64, 96, 320] f32.  GROUP=8, cpg=8, MAXDISP=48, D=49.
  out:  [2, 8, 49, 96, 320] f32
  out[b,g,k,h,j] = sum_c | xn[b,g,c,h,j] - yn_pad[b,g,c,h,j+48-k] |
  where xn/yn are channel-group L2-normalized (cpg=8) and yn_pad has 48 zero
  columns prepended along w.

Sharding: 16 independent (b, g) pairs -> 2 per core.

Per-core design (v5):
  SBUF partition p = c*16 + hh (c in 0..8, hh in 0..16); free = (ht, w),
  h = 16*ht + hh.  Disparities processed in TRIADS of 3 consecutive k
  (ascending within the triad via a negative-stride overlapping-window AP on
  the padded y) and a stride-0 broadcast of xn over the triad dim.

  Normalization: inputs stream in per 2-ht chunk (bg0 x on the SP hwdge
  queue in f32; everything else through Pool SWDGE DMAs that CAST f32->f16
  in flight), ACT squares -> one [128->128] replicated-channel-sum matmul
  (s0[r,p] = 1 iff r%16 == p%16, so no separate broadcast pass) -> ACT
  rsqrt -> DVE multiply at fp16 2x.  bg1's normalization is emitted one
  step per bg0 triad so it never head-of-line blocks the triad pipeline.

  Triad paths (DVE elementwise is the bottleneck: the real ISA rejects
  TensorTensor on Pool, so every min/sub runs on DVE at fp16 2x):
   'm' (14 of 17): DVE min; |a-b| = a+b-2min via per-k matmuls: id96 U +
        id96 V_k + 6 s6n(-2) channel-sum matmuls into one PSUM bank.
   'q' (3 of 17): DVE subtract -> ACT Abs in 2-ht chunks emitting fp8e4 ->
        3 PE DoubleRow matmuls per k (K=256, 0.5 cyc/row).
  Each triad accumulates into a [96, 3, 512] PSUM tile (2 bufs); ACT copies
  it to an fp16 [H, 3, W] tile; the store rides the ACT hwdge queue (deps
  retire just before it, so no SEQ-parked semaphore waits) and the pad
  strip [0:js) goes straight from the precomputed sum|xn| strip (a3) via a
  tiny second DMA on the idle SP queue.  Host converts fp16 -> f32.
"""

import numpy as np

B, C, H, W = 2, 64, 96, 320
GROUP = 8
CPG = C // GROUP          # 8
MAXDISP = 48
D = MAXDISP + 1           # 49
NCORES = 8
BG_TOTAL = B * GROUP      # 16
BG_PER_CORE = BG_TOTAL // NCORES  # 2
HH = 16
HT = H // HH              # 6
PW = 376                  # padded y width (48 zeros + 320 + 8 zero tail)
NG = 17                   # triads per bg (k=48..1), plus single k=0

# path per group index g=0..16 (g<16: triad kmax=48-3g; g=16: single k=0)
import os as _os
PATHS0 = list(_os.environ.get("CV_PATHS0", "mmqmmmmmqmmmqmmmm"))
PATHS1 = list(_os.environ.get("CV_PATHS1", "mmqmmmmmqmmmqmmmm"))
_PUMP = int(_os.environ.get("CV_PUMP", "1"))  # norm1 steps pumped per triad
_DPOOL = int(_os.environ.get("CV_DPOOL", "6"))
_CHUNK = int(_os.environ.get("CV_CHUNK", "2"))  # ht rows per norm chunk
_QCH = int(_os.environ.get("CV_QCH", "3"))      # abs chunks per q triad
_QF = int(_os.environ.get("CV_QF", "0"))        # 1: copy before abs
_OPOOL = int(_os.environ.get("CV_OPOOL", "8"))

_PROG = None


def _constants():
    import ml_dtypes

    # s0b[r, p] = 1 iff r % 16 == p % 16: sums the 8 channels and replicates
    # the result across all 128 partitions in one matmul (no broadcast pass)
    s0 = np.zeros((128, 128), np.float32)
    for p in range(128):
        for c in range(CPG):
            s0[c * HH + p % HH, p] = 1.0
    # s66[0:6] = +1 selectors (U/V/a3 prep), s66[6:12] = -2 selectors (min)
    s66 = np.zeros((2 * HT, 128, H), np.float16)
    for t in range(HT):
        for p in range(128):
            s66[t, p, HH * t + p % HH] = 1.0
            s66[HT + t, p, HH * t + p % HH] = -2.0
    s6dr = np.zeros((HT // 2, 128, 2, H), np.float32)
    for tp in range(HT // 2):
        for p in range(128):
            for i in range(2):
                s6dr[tp, p, i, 32 * tp + 16 * i + p % HH] = 1.0
    id96 = np.eye(H, dtype=np.float16)
    return {
        "s0f32": s0.astype(np.float16),
        "s66f16": s66,
        "s6dr8": s6dr.astype(ml_dtypes.float8_e4m3),
        "id96": id96,
    }


def _build():
    global _PROG
    if _PROG is not None:
        return _PROG

    import bass_rust
    import concourse.bacc as bacc
    import concourse.mybir as mybir
    import concourse.tile as tile

    VP = bass_rust.VecI64Pair
    f32 = mybir.dt.float32
    f32r = mybir.dt.float32r
    f16 = mybir.dt.float16
    f8e4 = mybir.dt.float8e4
    u16 = mybir.dt.uint16
    AF = mybir.ActivationFunctionType
    ALU = mybir.AluOpType
    PM = mybir.MatmulPerfMode

    nc = bacc.Bacc("TRN2", target_bir_lowering=False, debug=False)

    xin = nc.dram_tensor("x", [BG_PER_CORE, 128, HT, W], f32, kind="ExternalInput")
    yin = nc.dram_tensor("y", [BG_PER_CORE, 128, HT, W], f32, kind="ExternalInput")
    s0_d = nc.dram_tensor("s0f32", [128, 128], f16, kind="ExternalInput")
    s66_d = nc.dram_tensor("s66f16", [2 * HT, 128, H], f16, kind="ExternalInput")
    s6dr_d = nc.dram_tensor("s6dr8", [HT // 2, 128, 2, H], f8e4, kind="ExternalInput")
    id96_d = nc.dram_tensor("id96", [H, H], f16, kind="ExternalInput")
    out_d = nc.dram_tensor("out", [BG_PER_CORE, D, H, W], f16, kind="ExternalOutput")

    x_v = xin.ap()
    y_v = yin.ap()
    out_v = out_d.ap()

    def windows_desc(base_ap, last_start, nwin, width):
        """Overlapping windows on the last dim with DESCENDING start:
        window i reads cols [last_start - i, last_start - i + width)."""
        ap = base_ap[..., last_start:last_start + width].copy()
        dims = [tuple(v) for v in ap.ap]
        ap.ap = VP(dims[:-1] + [(-1, nwin), (1, width)])
        return ap

    with tile.TileContext(nc) as tc:
        with (
            tc.tile_pool(name="const", bufs=1) as constp,
            tc.tile_pool(name="stage", bufs=2) as stagep,
            tc.tile_pool(name="norm", bufs=2) as normp,
            tc.tile_pool(name="keep", bufs=1) as keepp,
            tc.tile_pool(name="nps", bufs=2, space="PSUM") as npsp,
            tc.tile_pool(name="mmps", bufs=2, space="PSUM") as mmpsp,
            tc.tile_pool(name="dpool", bufs=_DPOOL) as dpool,
            tc.tile_pool(name="qpool", bufs=3) as qpool,
            tc.tile_pool(name="opool", bufs=_OPOOL) as opool,
        ):
            # constants ride the idle Pool SWDGE queue, batched into 5 DMAs so
            # they don't occupy the Pool engine for long
            s0_t = constp.tile([128, 128], f16, tag="s0")
            id96 = constp.tile([H, H], f16, tag="id96")
            s66_t = constp.tile([128, 2 * HT, H], f16, tag="s66")
            s6dr_t = constp.tile([128, HT // 2, 2, H], f8e4, tag="s6dr")
            nc.gpsimd.dma_start(s0_t[:], s0_d.ap())
            consts_emitted = [False]

            def emit_weight_consts():
                if consts_emitted[0]:
                    return
                consts_emitted[0] = True
                nc.gpsimd.dma_start(id96[:], id96_d.ap())
                nc.gpsimd.dma_start(
                    s66_t[:], s66_d.ap().rearrange("t p h -> p t h"))
                nc.gpsimd.dma_start(
                    s6dr_t[:], s6dr_d.ap().rearrange("q p i h -> p q i h"))
            s6 = [s66_t[:, t] for t in range(HT)]
            s6n = [s66_t[:, HT + t] for t in range(HT)]
            s6dr = [s6dr_t[:, tp] for tp in range(HT // 2)]

            # persistent per-bg tiles
            xn, ynp, u1, vp, a3 = [], [], [], [], []
            for bg in range(BG_PER_CORE):
                xn.append(keepp.tile([128, HT, W], f16, tag=f"xn{bg}", name=f"xn{bg}"))
                ynp.append(keepp.tile([128, HT, PW], f16, tag=f"ynp{bg}", name=f"ynp{bg}"))
                u1.append(keepp.tile([H, W], f16, tag=f"u1{bg}", name=f"u1{bg}"))
                vp.append(keepp.tile([H, PW], f16, tag=f"vp{bg}", name=f"vp{bg}"))
                a3.append(keepp.tile([H, 3, 47], f16, tag=f"a3{bg}", name=f"a3{bg}"))

            # ---------------- normalization + U/V prep ----------------
            def emit_norm(bg):
                """Generator: emits normalization for one bg in ~10 steps."""
                nc.gpsimd.memset(ynp[bg][:, :, 0:MAXDISP], 0.0)
                nc.gpsimd.memset(ynp[bg][:, :, MAXDISP + W:], 0.0)
                raws = []
                sqs = []
                rss = []
                for is_y in (0, 1):
                    nm = "y" if is_y else "x"
                    raws.append(stagep.tile(
                        [128, HT, W], f32 if (bg == 0 and not is_y) else f16,
                        tag="raw", name=f"raw{nm}"))
                    sqs.append(stagep.tile(
                        [128, HT, W], f16, tag="sq", name=f"sq{nm}"))
                    rss.append(normp.tile(
                        [128, HT, W], f16, tag="rs", name=f"rs{nm}"))
                yield
                # pipelined per chunk: load -> square -> per-t ssum/rsqrt/mul
                for c0 in range(0, HT, _CHUNK):
                    sl = slice(c0, c0 + _CHUNK)
                    for is_y in (0, 1):
                        src_v = y_v if is_y else x_v
                        raw, sq, rs = raws[is_y], sqs[is_y], rss[is_y]
                        # f32 -> f16 cast happens inside the SWDGE DMA, so
                        # the norm multiply runs at DVE 2x on all-f16 operands.
                        # bg0's x rides SP uncast: Pool's serial desc-gen would
                        # otherwise pace the whole prologue.
                        if bg == 0 and not is_y:
                            nc.sync.dma_start(raw[:, sl], src_v[bg][:, sl])
                        else:
                            nc.gpsimd.dma_start(raw[:, sl], src_v[bg][:, sl])
                        nc.scalar.activation(
                            sq[:, sl].rearrange("p a b -> p (a b)"),
                            raw[:, sl].rearrange("p a b -> p (a b)"),
                            AF.Square,
                        )
                        for t in range(c0, c0 + _CHUNK):
                            ssum = npsp.tile([128, 512], f32, tag="ssum")
                            nc.tensor.matmul(
                                ssum[:, 0:W], s0_t[:], sq[:, t, :],
                                start=True, stop=True,
                            )
                            nc.scalar.activation(
                                rs[:, t, :], ssum[:, 0:W], AF.Abs_reciprocal_sqrt
                            )
                        if is_y:
                            nc.vector.tensor_mul(
                                ynp[bg][:, sl, MAXDISP:MAXDISP + W],
                                raw[:, sl], rs[:, sl],
                            )
                        else:
                            nc.vector.tensor_mul(
                                xn[bg][:, sl], raw[:, sl], rs[:, sl]
                            )
                        yield
                    if bg == 0 and c0 == 0:
                        emit_weight_consts()
                # U = sum_c xn -> [96, W]; V = sum_c ynp -> [96, PW]
                ups = npsp.tile([128, 512], f32, tag="ssum", name="ups")
                for t in range(HT):
                    nc.tensor.matmul(
                        ups[0:H, 0:W], s6[t], xn[bg][:, t, :],
                        start=(t == 0), stop=(t == HT - 1),
                    )
                nc.scalar.activation(u1[bg][:], ups[0:H, 0:W], AF.Copy)
                yield
                vps = npsp.tile([128, 512], f32, tag="ssum", name="vps")
                for t in range(HT):
                    nc.tensor.matmul(
                        vps[0:H, 0:PW], s6[t], ynp[bg][:, t, :],
                        start=(t == 0), stop=(t == HT - 1),
                    )
                nc.scalar.activation(vp[bg][:], vps[0:H, 0:PW], AF.Copy)
                yield
                # A3[h, kk, j] = sum_c |xn_c[h, j]| for j < 46: the pad-strip
                # output (identical for every k in a triad), stored 3-wide.
                ax = normp.tile([128, HT, 48], f16, tag="ax")
                nc.scalar.activation(
                    ax[:, :, 0:47], xn[bg][:, :, 0:47], AF.Abs,
                )
                aps = npsp.tile([128, 512], f32, tag="ssum", name="aps")
                for t in range(HT):
                    nc.tensor.matmul(
                        aps[0:H, 0:47], s6[t], ax[:, t, 0:47],
                        start=(t == 0), stop=(t == HT - 1),
                    )
                nc.scalar.activation(
                    a3[bg][:],
                    aps[0:H, 0:47].unsqueeze(1).broadcast_to([H, 3, 47]),
                    AF.Copy,
                )
                yield

            for _ in emit_norm(0):
                pass
            norm1 = emit_norm(1)

            # ---------------- main loop: triads ----------------
            # 1-triad software-pipeline skew: triad g's PSUM->SBUF copy and
            # store are emitted AFTER triad g+1's elementwise/abs work, so the
            # in-order ACT queue runs the next abs before stalling on the
            # copy's PSUM dependency.
            pending = None

            def flush_pending():
                nonlocal pending
                if pending is None:
                    return
                p_bg, p_ps, p_ob, p_nwin, p_kmax, p_js, p_path = pending
                w0 = W - p_js
                if p_path == "t":
                    nc.vector.tensor_scalar(
                        p_ob[:, 0:p_nwin, p_js:], p_ps[:, 0:p_nwin, 0:w0],
                        0.0, None, op0=ALU.add,
                    )
                else:
                    nc.scalar.activation(
                        p_ob[:, 0:p_nwin, p_js:], p_ps[:, 0:p_nwin, 0:w0],
                        AF.Copy,
                    )
                klo = p_kmax - p_nwin + 1
                # stores ride the ACT hwdge queue: their data deps retire just
                # before them there, so the SEQ isn't parked on long semaphore
                # waits (a single SP queue serialized the whole kernel on
                # those waits).  The pad strip [0:js) goes straight from a3
                # via a second tiny DMA -- no engine op at all.
                nc.scalar.dma_start(
                    out_v[p_bg, klo:p_kmax + 1, :, p_js:].rearrange(
                        "k h w -> h k w"),
                    p_ob[:, 0:p_nwin, p_js:],
                )
                if p_js > 0:
                    nc.sync.dma_start(
                        out_v[p_bg, klo:p_kmax + 1, :, 0:p_js].rearrange(
                            "k h w -> h k w"),
                        a3[p_bg][:, 0:p_nwin, 0:p_js],
                    )
                pending = None

            for bg in range(BG_PER_CORE):
                for g in range(NG):
                    path = (PATHS0 if bg == 0 else PATHS1)[g]
                    if g < NG - 1:
                        kmax = MAXDISP - 3 * g       # triad k = kmax-2..kmax
                        nwin = 3
                        base = MAXDISP - kmax        # 48 - k for k = kmax
                    else:
                        kmax = 0
                        nwin = 1
                        base = MAXDISP
                    js = kmax - nwin + 1             # valid window: j >= js
                    w0 = W - js
                    # window kk corresponds to k = js + kk (ascending):
                    # ynp col = (j - js) + base + js + (nwin-1) - kk
                    ywin = windows_desc(
                        ynp[bg][:], base + js + (nwin - 1), nwin, w0)
                    xin_ap = (
                        xn[bg][:, :, js:].unsqueeze(2)
                        .broadcast_to([128, HT, nwin, w0])
                    )

                    ps = mmpsp.tile([H, 3, 512], f32, tag="ps")
                    ob = opool.tile([H, 3, W], f16, tag="ob")
                    if path == "m":
                        tail = bg == BG_PER_CORE - 1 and g >= NG - 2
                        m3 = dpool.tile([128, HT, 3, W], f16, tag="d3", name="m3")
                        if tail:
                            # drain the pipeline: chunked mins let PE start
                            # before the whole min lands
                            for c0 in range(0, HT, 2):
                                nc.vector.tensor_tensor(
                                    m3[:, c0:c0 + 2, 0:nwin, 0:w0],
                                    xn[bg][:, c0:c0 + 2, js:].unsqueeze(2)
                                    .broadcast_to([128, 2, nwin, w0]),
                                    windows_desc(
                                        ynp[bg][:, c0:c0 + 2],
                                        base + js + (nwin - 1), nwin, w0),
                                    ALU.min,
                                )
                        else:
                            nc.vector.tensor_tensor(
                                m3[:, :, 0:nwin, 0:w0], xin_ap, ywin, ALU.min
                            )
                        flush_pending()
                        # per k: U + V_k + sum_c min (multi-bank matmul
                        # outputs fail the real ISA's s3d3 check, so one
                        # bank per matmul)
                        for kk in range(nwin):
                            vs = base + js + (nwin - 1) - kk
                            nc.tensor.matmul(
                                ps[:, kk, 0:w0], id96[:], u1[bg][:, js:],
                                start=True, stop=False,
                            )
                            nc.tensor.matmul(
                                ps[:, kk, 0:w0], id96[:],
                                vp[bg][:, vs:vs + w0],
                                start=False, stop=False,
                            )
                        for t in range(HT):
                            for kk in range(nwin):
                                nc.tensor.matmul(
                                    ps[:, kk, 0:w0], s6n[t],
                                    m3[:, t, kk, 0:w0],
                                    start=False, stop=(t == HT - 1),
                                )
                    else:  # 'q'
                        d3 = dpool.tile([128, HT, 3, W], f16, tag="d3", name="d3")
                        nc.vector.tensor_tensor(
                            d3[:, :, 0:nwin, 0:w0], xin_ap, ywin, ALU.subtract
                        )
                        q3 = qpool.tile([128, HT, 3, W], f8e4, tag="q3", name="q3")
                        # abs in per-2ht chunks so the PSUM-freeing copy of the
                        # previous group isn't head-of-line blocked behind one
                        # long ACT op
                        qch = HT // _QCH
                        if _QF:
                            flush_pending()
                        nc.scalar.activation(
                            q3[:, 0:qch, 0:nwin, 0:w0],
                            d3[:, 0:qch, 0:nwin, 0:w0],
                            AF.Abs,
                        )
                        if not _QF:
                            flush_pending()
                        for ci in range(1, _QCH):
                            nc.scalar.activation(
                                q3[:, ci * qch:(ci + 1) * qch, 0:nwin, 0:w0],
                                d3[:, ci * qch:(ci + 1) * qch, 0:nwin, 0:w0],
                                AF.Abs,
                            )
                        for kk in range(nwin):
                            for tp in range(HT // 2):
                                nc.tensor.matmul(
                                    ps[:, kk, 0:w0], s6dr[tp],
                                    q3[:, 2 * tp:2 * tp + 2, kk, 0:w0],
                                    start=(tp == 0), stop=(tp == HT // 2 - 1),
                                    perf_mode=PM.DoubleRow,
                                )

                    pending = (bg, ps, ob, nwin, kmax, js, path)
                    if bg == 0:
                        for _ in range(_PUMP):
                            next(norm1, None)
                if bg == 0:
                    for _ in norm1:
                        pass
            flush_pending()

    nc.compile()
    _PROG = nc
    return nc


def run(x, y, trace=False, trace_kwargs=None):
    """x, y: full [2, 64, 96, 320] f32. Returns (out [2,8,49,96,320] f32, res)."""
    from concourse import bass_utils

    nc = _build()
    consts = _constants()

    def _prep(a):
        a = np.asarray(a, np.float32).reshape(BG_TOTAL, CPG, HT, HH, W)
        return np.ascontiguousarray(a.transpose(0, 1, 3, 2, 4)).reshape(
            BG_TOTAL, 128, HT, W
        )

    xr = _prep(x)
    yr = _prep(y)
    in_maps = []
    for i in range(NCORES):
        sl = slice(i * BG_PER_CORE, (i + 1) * BG_PER_CORE)
        in_maps.append(
            {
                "x": np.ascontiguousarray(xr[sl]),
                "y": np.ascontiguousarray(yr[sl]),
                **consts,
            }
        )
    res = bass_utils.run_bass_kernel_spmd(
        nc,
        in_maps,
        core_ids=list(range(NCORES)),
        trace=trace,
        **(trace_kwargs or {}),
    )
    full = np.concatenate(
        [np.asarray(r["out"], np.float32) for r in res.results], axis=0
    )
    out = full.reshape(B, GROUP, D, H, W)
    return out, res


def kernel(x, y):
    out, _ = run(x, y, trace=False)
    return out


# revision 34
# speedup vs baseline: 1.0150x; 1.0135x over previous
"""CostVolume Trainium2 kernel (Bass/Tile), SPMD over 8 NeuronCores.

Problem (hardcoded shapes):
  x, y: [2, 64, 96, 320] f32.  GROUP=8, cpg=8, MAXDISP=48, D=49.
  out:  [2, 8, 49, 96, 320] f32
  out[b,g,k,h,j] = sum_c | xn[b,g,c,h,j] - yn_pad[b,g,c,h,j+48-k] |
  where xn/yn are channel-group L2-normalized (cpg=8) and yn_pad has 48 zero
  columns prepended along w.

Sharding: 16 independent (b, g) pairs -> 2 per core.

Per-core design (v5):
  SBUF partition p = c*16 + hh (c in 0..8, hh in 0..16); free = (ht, w),
  h = 16*ht + hh.  Disparities processed in TRIADS of 3 consecutive k
  (ascending within the triad via a negative-stride overlapping-window AP on
  the padded y) and a stride-0 broadcast of xn over the triad dim.

  Normalization: inputs stream in per 2-ht chunk (bg0 x on the SP hwdge
  queue in f32; everything else through Pool SWDGE DMAs that CAST f32->f16
  in flight), ACT squares -> one [128->128] replicated-channel-sum matmul
  (s0[r,p] = 1 iff r%16 == p%16, so no separate broadcast pass) -> ACT
  rsqrt -> DVE multiply at fp16 2x.  bg1's normalization is emitted one
  step per bg0 triad so it never head-of-line blocks the triad pipeline.

  Triad paths (DVE elementwise is the bottleneck: the real ISA rejects
  TensorTensor on Pool, so every min/sub runs on DVE at fp16 2x):
   'm' (14 of 17): DVE min; |a-b| = a+b-2min via per-k matmuls: id96 U +
        id96 V_k + 6 s6n(-2) channel-sum matmuls into one PSUM bank.
   'q' (3 of 17): DVE subtract -> ACT Abs in 2-ht chunks emitting fp8e4 ->
        3 PE DoubleRow matmuls per k (K=256, 0.5 cyc/row).
  Each triad accumulates into a [96, 3, 512] PSUM tile (2 bufs); ACT copies
  it to an fp16 [H, 3, W] tile; the store rides the ACT hwdge queue (deps
  retire just before it, so no SEQ-parked semaphore waits) and the pad
  strip [0:js) goes straight from the precomputed sum|xn| strip (a3) via a
  tiny second DMA on the idle SP queue.  Host converts fp16 -> f32.
"""

import numpy as np

B, C, H, W = 2, 64, 96, 320
GROUP = 8
CPG = C // GROUP          # 8
MAXDISP = 48
D = MAXDISP + 1           # 49
NCORES = 8
BG_TOTAL = B * GROUP      # 16
BG_PER_CORE = BG_TOTAL // NCORES  # 2
HH = 16
HT = H // HH              # 6
PW = 376                  # padded y width (48 zeros + 320 + 8 zero tail)
NG = 17                   # triads per bg (k=48..1), plus single k=0

# path per group index g=0..16 (g<16: triad kmax=48-3g; g=16: single k=0)
import os as _os
PATHS0 = list(_os.environ.get("CV_PATHS0", "mmqmmmmmqmmmqmmmm"))
PATHS1 = list(_os.environ.get("CV_PATHS1", "mmqmmmmmqmmmqmmmm"))
_PUMP = int(_os.environ.get("CV_PUMP", "1"))  # norm1 steps pumped per triad
_DPOOL = int(_os.environ.get("CV_DPOOL", "6"))
_CHUNK = int(_os.environ.get("CV_CHUNK", "2"))  # ht rows per norm chunk
_QCH = int(_os.environ.get("CV_QCH", "3"))      # abs chunks per q triad
_QF = int(_os.environ.get("CV_QF", "0"))        # 1: copy before abs
_OPOOL = int(_os.environ.get("CV_OPOOL", "8"))

_PROG = None


def _constants():
    import ml_dtypes

    # s0b[r, p] = 1 iff r % 16 == p % 16: sums the 8 channels and replicates
    # the result across all 128 partitions in one matmul (no broadcast pass)
    s0 = np.zeros((128, 128), np.float32)
    for p in range(128):
        for c in range(CPG):
            s0[c * HH + p % HH, p] = 1.0
    # s66[0:6] = +1 selectors (U/V/a3 prep), s66[6:12] = -2 selectors (min)
    s66 = np.zeros((2 * HT, 128, H), np.float16)
    for t in range(HT):
        for p in range(128):
            s66[t, p, HH * t + p % HH] = 1.0
            s66[HT + t, p, HH * t + p % HH] = -2.0
    s6dr = np.zeros((HT // 2, 128, 2, H), np.float32)
    for tp in range(HT // 2):
        for p in range(128):
            for i in range(2):
                s6dr[tp, p, i, 32 * tp + 16 * i + p % HH] = 1.0
    id96 = np.eye(H, dtype=np.float16)
    return {
        "s0f32": s0.astype(np.float16),
        "s66f16": s66,
        "s6dr8": s6dr.astype(ml_dtypes.float8_e4m3),
        "id96": id96,
    }


def _build():
    global _PROG
    if _PROG is not None:
        return _PROG

    import bass_rust
    import concourse.bacc as bacc
    import concourse.mybir as mybir
    import concourse.tile as tile

    VP = bass_rust.VecI64Pair
    f32 = mybir.dt.float32
    f32r = mybir.dt.float32r
    f16 = mybir.dt.float16
    f8e4 = mybir.dt.float8e4
    u16 = mybir.dt.uint16
    AF = mybir.ActivationFunctionType
    ALU = mybir.AluOpType
    PM = mybir.MatmulPerfMode

    nc = bacc.Bacc("TRN2", target_bir_lowering=False, debug=False)

    xin = nc.dram_tensor("x", [BG_PER_CORE, 128, HT, W], f32, kind="ExternalInput")
    yin = nc.dram_tensor("y", [BG_PER_CORE, 128, HT, W], f32, kind="ExternalInput")
    s0_d = nc.dram_tensor("s0f32", [128, 128], f16, kind="ExternalInput")
    s66_d = nc.dram_tensor("s66f16", [2 * HT, 128, H], f16, kind="ExternalInput")
    s6dr_d = nc.dram_tensor("s6dr8", [HT // 2, 128, 2, H], f8e4, kind="ExternalInput")
    id96_d = nc.dram_tensor("id96", [H, H], f16, kind="ExternalInput")
    out_d = nc.dram_tensor("out", [BG_PER_CORE, D, H, W], f16, kind="ExternalOutput")

    x_v = xin.ap()
    y_v = yin.ap()
    out_v = out_d.ap()

    def windows_desc(base_ap, last_start, nwin, width):
        """Overlapping windows on the last dim with DESCENDING start:
        window i reads cols [last_start - i, last_start - i + width)."""
        ap = base_ap[..., last_start:last_start + width].copy()
        dims = [tuple(v) for v in ap.ap]
        ap.ap = VP(dims[:-1] + [(-1, nwin), (1, width)])
        return ap

    with tile.TileContext(nc) as tc:
        with (
            tc.tile_pool(name="const", bufs=1) as constp,
            tc.tile_pool(name="stage", bufs=2) as stagep,
            tc.tile_pool(name="norm", bufs=2) as normp,
            tc.tile_pool(name="keep", bufs=1) as keepp,
            tc.tile_pool(name="nps", bufs=2, space="PSUM") as npsp,
            tc.tile_pool(name="mmps", bufs=2, space="PSUM") as mmpsp,
            tc.tile_pool(name="dpool", bufs=_DPOOL) as dpool,
            tc.tile_pool(name="qpool", bufs=3) as qpool,
            tc.tile_pool(name="opool", bufs=_OPOOL) as opool,
        ):
            # constants ride the idle Pool SWDGE queue, batched into 5 DMAs so
            # they don't occupy the Pool engine for long
            s0_t = constp.tile([128, 128], f16, tag="s0")
            id96 = constp.tile([H, H], f16, tag="id96")
            s66_t = constp.tile([128, 2 * HT, H], f16, tag="s66")
            s6dr_t = constp.tile([128, HT // 2, 2, H], f8e4, tag="s6dr")
            nc.gpsimd.dma_start(s0_t[:], s0_d.ap())
            consts_emitted = [False]

            def emit_weight_consts():
                if consts_emitted[0]:
                    return
                consts_emitted[0] = True
                nc.gpsimd.dma_start(id96[:], id96_d.ap())
                nc.gpsimd.dma_start(
                    s66_t[:], s66_d.ap().rearrange("t p h -> p t h"))
                nc.gpsimd.dma_start(
                    s6dr_t[:], s6dr_d.ap().rearrange("q p i h -> p q i h"))
            s6 = [s66_t[:, t] for t in range(HT)]
            s6n = [s66_t[:, HT + t] for t in range(HT)]
            s6dr = [s6dr_t[:, tp] for tp in range(HT // 2)]

            # persistent per-bg tiles
            xn, ynp, u1, vp, a3 = [], [], [], [], []
            for bg in range(BG_PER_CORE):
                xn.append(keepp.tile([128, HT, W], f16, tag=f"xn{bg}", name=f"xn{bg}"))
                ynp.append(keepp.tile([128, HT, PW], f16, tag=f"ynp{bg}", name=f"ynp{bg}"))
                u1.append(keepp.tile([H, W], f16, tag=f"u1{bg}", name=f"u1{bg}"))
                vp.append(keepp.tile([H, PW], f16, tag=f"vp{bg}", name=f"vp{bg}"))
                a3.append(keepp.tile([H, 3, 47], f16, tag=f"a3{bg}", name=f"a3{bg}"))

            # ---------------- normalization + U/V prep ----------------
            def emit_norm(bg):
                """Generator: emits normalization for one bg in ~10 steps."""
                nc.gpsimd.memset(ynp[bg][:, :, 0:MAXDISP], 0.0)
                nc.gpsimd.memset(ynp[bg][:, :, MAXDISP + W:], 0.0)
                raws = []
                sqs = []
                rss = []
                for is_y in (0, 1):
                    nm = "y" if is_y else "x"
                    raws.append(stagep.tile(
                        [128, HT, W], f32 if (bg == 0 and not is_y) else f16,
                        tag="raw", name=f"raw{nm}"))
                    sqs.append(stagep.tile(
                        [128, HT, W], f16, tag="sq", name=f"sq{nm}"))
                    rss.append(normp.tile(
                        [128, HT, W], f16, tag="rs", name=f"rs{nm}"))
                yield
                # pipelined per chunk: load -> square -> per-t ssum/rsqrt/mul
                for c0 in range(0, HT, _CHUNK):
                    sl = slice(c0, c0 + _CHUNK)
                    for is_y in (0, 1):
                        src_v = y_v if is_y else x_v
                        raw, sq, rs = raws[is_y], sqs[is_y], rss[is_y]
                        # f32 -> f16 cast happens inside the SWDGE DMA, so
                        # the norm multiply runs at DVE 2x on all-f16 operands.
                        # bg0's x rides SP uncast: Pool's serial desc-gen would
                        # otherwise pace the whole prologue.
                        if bg == 0 and not is_y:
                            nc.sync.dma_start(raw[:, sl], src_v[bg][:, sl])
                        else:
                            nc.gpsimd.dma_start(raw[:, sl], src_v[bg][:, sl])
                        nc.scalar.activation(
                            sq[:, sl].rearrange("p a b -> p (a b)"),
                            raw[:, sl].rearrange("p a b -> p (a b)"),
                            AF.Square,
                        )
                        for t in range(c0, c0 + _CHUNK):
                            ssum = npsp.tile([128, 512], f32, tag="ssum")
                            nc.tensor.matmul(
                                ssum[:, 0:W], s0_t[:], sq[:, t, :],
                                start=True, stop=True,
                            )
                            nc.scalar.activation(
                                rs[:, t, :], ssum[:, 0:W], AF.Abs_reciprocal_sqrt
                            )
                        if is_y:
                            nc.vector.tensor_mul(
                                ynp[bg][:, sl, MAXDISP:MAXDISP + W],
                                raw[:, sl], rs[:, sl],
                            )
                        else:
                            nc.vector.tensor_mul(
                                xn[bg][:, sl], raw[:, sl], rs[:, sl]
                            )
                        yield
                    if bg == 0 and c0 == 0:
                        emit_weight_consts()
                    if bg == 0:
                        head_min_chunk(c0, _CHUNK)
                # U = sum_c xn -> [96, W]; V = sum_c ynp -> [96, PW]
                ups = npsp.tile([128, 512], f32, tag="ssum", name="ups")
                for t in range(HT):
                    nc.tensor.matmul(
                        ups[0:H, 0:W], s6[t], xn[bg][:, t, :],
                        start=(t == 0), stop=(t == HT - 1),
                    )
                nc.scalar.activation(u1[bg][:], ups[0:H, 0:W], AF.Copy)
                yield
                vps = npsp.tile([128, 512], f32, tag="ssum", name="vps")
                for t in range(HT):
                    nc.tensor.matmul(
                        vps[0:H, 0:PW], s6[t], ynp[bg][:, t, :],
                        start=(t == 0), stop=(t == HT - 1),
                    )
                nc.scalar.activation(vp[bg][:], vps[0:H, 0:PW], AF.Copy)
                yield
                # A3[h, kk, j] = sum_c |xn_c[h, j]| for j < 46: the pad-strip
                # output (identical for every k in a triad), stored 3-wide.
                ax = normp.tile([128, HT, 48], f16, tag="ax")
                nc.scalar.activation(
                    ax[:, :, 0:47], xn[bg][:, :, 0:47], AF.Abs,
                )
                aps = npsp.tile([128, 512], f32, tag="ssum", name="aps")
                for t in range(HT):
                    nc.tensor.matmul(
                        aps[0:H, 0:47], s6[t], ax[:, t, 0:47],
                        start=(t == 0), stop=(t == HT - 1),
                    )
                nc.scalar.activation(
                    a3[bg][:],
                    aps[0:H, 0:47].unsqueeze(1).broadcast_to([H, 3, 47]),
                    AF.Copy,
                )
                yield

            # the first two triads' mins only need the xn/ynp rows of each
            # finished norm chunk, so they are emitted chunk-by-chunk inside
            # norm0 and fill DVE's otherwise idle prologue
            head_m3 = {}

            def head_min_chunk(c0, ch):
                for g in (0, 1):
                    if PATHS0[g] != "m":
                        continue
                    if g not in head_m3:
                        head_m3[g] = dpool.tile(
                            [128, HT, 3, W], f16, tag="d3", name="m3")
                    kmax = MAXDISP - 3 * g
                    base = MAXDISP - kmax
                    js = kmax - 2
                    w0 = W - js
                    nc.vector.tensor_tensor(
                        head_m3[g][:, c0:c0 + ch, 0:3, 0:w0],
                        xn[0][:, c0:c0 + ch, js:].unsqueeze(2)
                        .broadcast_to([128, ch, 3, w0]),
                        windows_desc(
                            ynp[0][:, c0:c0 + ch], base + js + 2, 3, w0),
                        ALU.min,
                    )

            for _ in emit_norm(0):
                pass
            norm1 = emit_norm(1)

            # ---------------- main loop: triads ----------------
            # 1-triad software-pipeline skew: triad g's PSUM->SBUF copy and
            # store are emitted AFTER triad g+1's elementwise/abs work, so the
            # in-order ACT queue runs the next abs before stalling on the
            # copy's PSUM dependency.
            pending = None

            def flush_pending():
                nonlocal pending
                if pending is None:
                    return
                p_bg, p_ps, p_ob, p_nwin, p_kmax, p_js, p_path = pending
                w0 = W - p_js
                if p_path == "t":
                    nc.vector.tensor_scalar(
                        p_ob[:, 0:p_nwin, p_js:], p_ps[:, 0:p_nwin, 0:w0],
                        0.0, None, op0=ALU.add,
                    )
                else:
                    nc.scalar.activation(
                        p_ob[:, 0:p_nwin, p_js:], p_ps[:, 0:p_nwin, 0:w0],
                        AF.Copy,
                    )
                klo = p_kmax - p_nwin + 1
                # stores ride the ACT hwdge queue: their data deps retire just
                # before them there, so the SEQ isn't parked on long semaphore
                # waits (a single SP queue serialized the whole kernel on
                # those waits).  The pad strip [0:js) goes straight from a3
                # via a second tiny DMA -- no engine op at all.
                nc.scalar.dma_start(
                    out_v[p_bg, klo:p_kmax + 1, :, p_js:].rearrange(
                        "k h w -> h k w"),
                    p_ob[:, 0:p_nwin, p_js:],
                )
                if p_js > 0:
                    nc.sync.dma_start(
                        out_v[p_bg, klo:p_kmax + 1, :, 0:p_js].rearrange(
                            "k h w -> h k w"),
                        a3[p_bg][:, 0:p_nwin, 0:p_js],
                    )
                pending = None

            for bg in range(BG_PER_CORE):
                for g in range(NG):
                    path = (PATHS0 if bg == 0 else PATHS1)[g]
                    if g < NG - 1:
                        kmax = MAXDISP - 3 * g       # triad k = kmax-2..kmax
                        nwin = 3
                        base = MAXDISP - kmax        # 48 - k for k = kmax
                    else:
                        kmax = 0
                        nwin = 1
                        base = MAXDISP
                    js = kmax - nwin + 1             # valid window: j >= js
                    w0 = W - js
                    # window kk corresponds to k = js + kk (ascending):
                    # ynp col = (j - js) + base + js + (nwin-1) - kk
                    ywin = windows_desc(
                        ynp[bg][:], base + js + (nwin - 1), nwin, w0)
                    xin_ap = (
                        xn[bg][:, :, js:].unsqueeze(2)
                        .broadcast_to([128, HT, nwin, w0])
                    )

                    ps = mmpsp.tile([H, 3, 512], f32, tag="ps")
                    ob = opool.tile([H, 3, W], f16, tag="ob")
                    if path == "m":
                        tail = bg == BG_PER_CORE - 1 and g >= NG - 2
                        if bg == 0 and g in head_m3:
                            m3 = head_m3[g]
                        else:
                            m3 = dpool.tile(
                                [128, HT, 3, W], f16, tag="d3", name="m3")
                        if bg == 0 and g in head_m3:
                            pass
                        elif tail:
                            # drain the pipeline: chunked mins let PE start
                            # before the whole min lands
                            for c0 in range(0, HT, 2):
                                nc.vector.tensor_tensor(
                                    m3[:, c0:c0 + 2, 0:nwin, 0:w0],
                                    xn[bg][:, c0:c0 + 2, js:].unsqueeze(2)
                                    .broadcast_to([128, 2, nwin, w0]),
                                    windows_desc(
                                        ynp[bg][:, c0:c0 + 2],
                                        base + js + (nwin - 1), nwin, w0),
                                    ALU.min,
                                )
                        else:
                            nc.vector.tensor_tensor(
                                m3[:, :, 0:nwin, 0:w0], xin_ap, ywin, ALU.min
                            )
                        flush_pending()
                        # per k: U + V_k + sum_c min (multi-bank matmul
                        # outputs fail the real ISA's s3d3 check, so one
                        # bank per matmul)
                        for kk in range(nwin):
                            vs = base + js + (nwin - 1) - kk
                            nc.tensor.matmul(
                                ps[:, kk, 0:w0], id96[:], u1[bg][:, js:],
                                start=True, stop=False,
                            )
                            nc.tensor.matmul(
                                ps[:, kk, 0:w0], id96[:],
                                vp[bg][:, vs:vs + w0],
                                start=False, stop=False,
                            )
                        for t in range(HT):
                            for kk in range(nwin):
                                nc.tensor.matmul(
                                    ps[:, kk, 0:w0], s6n[t],
                                    m3[:, t, kk, 0:w0],
                                    start=False, stop=(t == HT - 1),
                                )
                    else:  # 'q'
                        d3 = dpool.tile([128, HT, 3, W], f16, tag="d3", name="d3")
                        nc.vector.tensor_tensor(
                            d3[:, :, 0:nwin, 0:w0], xin_ap, ywin, ALU.subtract
                        )
                        q3 = qpool.tile([128, HT, 3, W], f8e4, tag="q3", name="q3")
                        # abs in per-2ht chunks so the PSUM-freeing copy of the
                        # previous group isn't head-of-line blocked behind one
                        # long ACT op
                        qch = HT // _QCH
                        if _QF:
                            flush_pending()
                        nc.scalar.activation(
                            q3[:, 0:qch, 0:nwin, 0:w0],
                            d3[:, 0:qch, 0:nwin, 0:w0],
                            AF.Abs,
                        )
                        if not _QF:
                            flush_pending()
                        for ci in range(1, _QCH):
                            nc.scalar.activation(
                                q3[:, ci * qch:(ci + 1) * qch, 0:nwin, 0:w0],
                                d3[:, ci * qch:(ci + 1) * qch, 0:nwin, 0:w0],
                                AF.Abs,
                            )
                        for kk in range(nwin):
                            for tp in range(HT // 2):
                                nc.tensor.matmul(
                                    ps[:, kk, 0:w0], s6dr[tp],
                                    q3[:, 2 * tp:2 * tp + 2, kk, 0:w0],
                                    start=(tp == 0), stop=(tp == HT // 2 - 1),
                                    perf_mode=PM.DoubleRow,
                                )

                    pending = (bg, ps, ob, nwin, kmax, js, path)
                    if bg == 0:
                        for _ in range(_PUMP):
                            next(norm1, None)
                if bg == 0:
                    for _ in norm1:
                        pass
            flush_pending()

    nc.compile()
    _PROG = nc
    return nc


def run(x, y, trace=False, trace_kwargs=None):
    """x, y: full [2, 64, 96, 320] f32. Returns (out [2,8,49,96,320] f32, res)."""
    from concourse import bass_utils

    nc = _build()
    consts = _constants()

    def _prep(a):
        a = np.asarray(a, np.float32).reshape(BG_TOTAL, CPG, HT, HH, W)
        return np.ascontiguousarray(a.transpose(0, 1, 3, 2, 4)).reshape(
            BG_TOTAL, 128, HT, W
        )

    xr = _prep(x)
    yr = _prep(y)
    in_maps = []
    for i in range(NCORES):
        sl = slice(i * BG_PER_CORE, (i + 1) * BG_PER_CORE)
        in_maps.append(
            {
                "x": np.ascontiguousarray(xr[sl]),
                "y": np.ascontiguousarray(yr[sl]),
                **consts,
            }
        )
    res = bass_utils.run_bass_kernel_spmd(
        nc,
        in_maps,
        core_ids=list(range(NCORES)),
        trace=trace,
        **(trace_kwargs or {}),
    )
    full = np.concatenate(
        [np.asarray(r["out"], np.float32) for r in res.results], axis=0
    )
    out = full.reshape(B, GROUP, D, H, W)
    return out, res


def kernel(x, y):
    out, _ = run(x, y, trace=False)
    return out


# revision 35
# speedup vs baseline: 1.0154x; 1.0005x over previous
"""CostVolume Trainium2 kernel (Bass/Tile), SPMD over 8 NeuronCores.

Problem (hardcoded shapes):
  x, y: [2, 64, 96, 320] f32.  GROUP=8, cpg=8, MAXDISP=48, D=49.
  out:  [2, 8, 49, 96, 320] f32
  out[b,g,k,h,j] = sum_c | xn[b,g,c,h,j] - yn_pad[b,g,c,h,j+48-k] |
  where xn/yn are channel-group L2-normalized (cpg=8) and yn_pad has 48 zero
  columns prepended along w.

Sharding: 16 independent (b, g) pairs -> 2 per core.

Per-core design (v5):
  SBUF partition p = c*16 + hh (c in 0..8, hh in 0..16); free = (ht, w),
  h = 16*ht + hh.  Disparities processed in TRIADS of 3 consecutive k
  (ascending within the triad via a negative-stride overlapping-window AP on
  the padded y) and a stride-0 broadcast of xn over the triad dim.

  Normalization: inputs stream in per 2-ht chunk (bg0 x on the SP hwdge
  queue in f32; everything else through Pool SWDGE DMAs that CAST f32->f16
  in flight), ACT squares -> one [128->128] replicated-channel-sum matmul
  (s0[r,p] = 1 iff r%16 == p%16, so no separate broadcast pass) -> ACT
  rsqrt -> DVE multiply at fp16 2x.  bg1's normalization is emitted one
  step per bg0 triad so it never head-of-line blocks the triad pipeline.

  Triad paths (DVE elementwise is the bottleneck: the real ISA rejects
  TensorTensor on Pool, so every min/sub runs on DVE at fp16 2x):
   'm' (14 of 17): DVE min; |a-b| = a+b-2min via per-k matmuls: id96 U +
        id96 V_k + 6 s6n(-2) channel-sum matmuls into one PSUM bank.
   'q' (3 of 17): DVE subtract -> ACT Abs in 2-ht chunks emitting fp8e4 ->
        3 PE DoubleRow matmuls per k (K=256, 0.5 cyc/row).
  Each triad accumulates into a [96, 3, 512] PSUM tile (2 bufs); ACT copies
  it to an fp16 [H, 3, W] tile; the store rides the ACT hwdge queue (deps
  retire just before it, so no SEQ-parked semaphore waits) and the pad
  strip [0:js) goes straight from the precomputed sum|xn| strip (a3) via a
  tiny second DMA on the idle SP queue.  Host converts fp16 -> f32.
"""

import numpy as np

B, C, H, W = 2, 64, 96, 320
GROUP = 8
CPG = C // GROUP          # 8
MAXDISP = 48
D = MAXDISP + 1           # 49
NCORES = 8
BG_TOTAL = B * GROUP      # 16
BG_PER_CORE = BG_TOTAL // NCORES  # 2
HH = 16
HT = H // HH              # 6
PW = 376                  # padded y width (48 zeros + 320 + 8 zero tail)
NG = 17                   # triads per bg (k=48..1), plus single k=0

# path per group index g=0..16 (g<16: triad kmax=48-3g; g=16: single k=0)
import os as _os
PATHS0 = list(_os.environ.get("CV_PATHS0", "mmqmmmmmqmmmqmmmm"))
PATHS1 = list(_os.environ.get("CV_PATHS1", "mmqmmmmmqmmmmqmmm"))
_PUMP = int(_os.environ.get("CV_PUMP", "1"))  # norm1 steps pumped per triad
_DPOOL = int(_os.environ.get("CV_DPOOL", "6"))
_CHUNK = int(_os.environ.get("CV_CHUNK", "2"))  # ht rows per norm chunk
_QCH = int(_os.environ.get("CV_QCH", "3"))      # abs chunks per q triad
_QF = int(_os.environ.get("CV_QF", "0"))        # 1: copy before abs
_OPOOL = int(_os.environ.get("CV_OPOOL", "8"))

_PROG = None


def _constants():
    import ml_dtypes

    # s0b[r, p] = 1 iff r % 16 == p % 16: sums the 8 channels and replicates
    # the result across all 128 partitions in one matmul (no broadcast pass)
    s0 = np.zeros((128, 128), np.float32)
    for p in range(128):
        for c in range(CPG):
            s0[c * HH + p % HH, p] = 1.0
    # s66[0:6] = +1 selectors (U/V/a3 prep), s66[6:12] = -2 selectors (min)
    s66 = np.zeros((2 * HT, 128, H), np.float16)
    for t in range(HT):
        for p in range(128):
            s66[t, p, HH * t + p % HH] = 1.0
            s66[HT + t, p, HH * t + p % HH] = -2.0
    s6dr = np.zeros((HT // 2, 128, 2, H), np.float32)
    for tp in range(HT // 2):
        for p in range(128):
            for i in range(2):
                s6dr[tp, p, i, 32 * tp + 16 * i + p % HH] = 1.0
    id96 = np.eye(H, dtype=np.float16)
    return {
        "s0f32": s0.astype(np.float16),
        "s66f16": s66,
        "s6dr8": s6dr.astype(ml_dtypes.float8_e4m3),
        "id96": id96,
    }


def _build():
    global _PROG
    if _PROG is not None:
        return _PROG

    import bass_rust
    import concourse.bacc as bacc
    import concourse.mybir as mybir
    import concourse.tile as tile

    VP = bass_rust.VecI64Pair
    f32 = mybir.dt.float32
    f32r = mybir.dt.float32r
    f16 = mybir.dt.float16
    f8e4 = mybir.dt.float8e4
    u16 = mybir.dt.uint16
    AF = mybir.ActivationFunctionType
    ALU = mybir.AluOpType
    PM = mybir.MatmulPerfMode

    nc = bacc.Bacc("TRN2", target_bir_lowering=False, debug=False)

    xin = nc.dram_tensor("x", [BG_PER_CORE, 128, HT, W], f32, kind="ExternalInput")
    yin = nc.dram_tensor("y", [BG_PER_CORE, 128, HT, W], f32, kind="ExternalInput")
    s0_d = nc.dram_tensor("s0f32", [128, 128], f16, kind="ExternalInput")
    s66_d = nc.dram_tensor("s66f16", [2 * HT, 128, H], f16, kind="ExternalInput")
    s6dr_d = nc.dram_tensor("s6dr8", [HT // 2, 128, 2, H], f8e4, kind="ExternalInput")
    id96_d = nc.dram_tensor("id96", [H, H], f16, kind="ExternalInput")
    out_d = nc.dram_tensor("out", [BG_PER_CORE, D, H, W], f16, kind="ExternalOutput")

    x_v = xin.ap()
    y_v = yin.ap()
    out_v = out_d.ap()

    def windows_desc(base_ap, last_start, nwin, width):
        """Overlapping windows on the last dim with DESCENDING start:
        window i reads cols [last_start - i, last_start - i + width)."""
        ap = base_ap[..., last_start:last_start + width].copy()
        dims = [tuple(v) for v in ap.ap]
        ap.ap = VP(dims[:-1] + [(-1, nwin), (1, width)])
        return ap

    with tile.TileContext(nc) as tc:
        with (
            tc.tile_pool(name="const", bufs=1) as constp,
            tc.tile_pool(name="stage", bufs=2) as stagep,
            tc.tile_pool(name="norm", bufs=2) as normp,
            tc.tile_pool(name="keep", bufs=1) as keepp,
            tc.tile_pool(name="nps", bufs=2, space="PSUM") as npsp,
            tc.tile_pool(name="mmps", bufs=2, space="PSUM") as mmpsp,
            tc.tile_pool(name="dpool", bufs=_DPOOL) as dpool,
            tc.tile_pool(name="qpool", bufs=3) as qpool,
            tc.tile_pool(name="opool", bufs=_OPOOL) as opool,
        ):
            # constants ride the idle Pool SWDGE queue, batched into 5 DMAs so
            # they don't occupy the Pool engine for long
            s0_t = constp.tile([128, 128], f16, tag="s0")
            id96 = constp.tile([H, H], f16, tag="id96")
            s66_t = constp.tile([128, 2 * HT, H], f16, tag="s66")
            s6dr_t = constp.tile([128, HT // 2, 2, H], f8e4, tag="s6dr")
            nc.gpsimd.dma_start(s0_t[:], s0_d.ap())
            consts_emitted = [False]

            def emit_weight_consts():
                if consts_emitted[0]:
                    return
                consts_emitted[0] = True
                nc.gpsimd.dma_start(id96[:], id96_d.ap())
                nc.gpsimd.dma_start(
                    s66_t[:], s66_d.ap().rearrange("t p h -> p t h"))
                nc.gpsimd.dma_start(
                    s6dr_t[:], s6dr_d.ap().rearrange("q p i h -> p q i h"))
            s6 = [s66_t[:, t] for t in range(HT)]
            s6n = [s66_t[:, HT + t] for t in range(HT)]
            s6dr = [s6dr_t[:, tp] for tp in range(HT // 2)]

            # persistent per-bg tiles
            xn, ynp, u1, vp, a3 = [], [], [], [], []
            for bg in range(BG_PER_CORE):
                xn.append(keepp.tile([128, HT, W], f16, tag=f"xn{bg}", name=f"xn{bg}"))
                ynp.append(keepp.tile([128, HT, PW], f16, tag=f"ynp{bg}", name=f"ynp{bg}"))
                u1.append(keepp.tile([H, W], f16, tag=f"u1{bg}", name=f"u1{bg}"))
                vp.append(keepp.tile([H, PW], f16, tag=f"vp{bg}", name=f"vp{bg}"))
                a3.append(keepp.tile([H, 3, 47], f16, tag=f"a3{bg}", name=f"a3{bg}"))

            # ---------------- normalization + U/V prep ----------------
            def emit_norm(bg):
                """Generator: emits normalization for one bg in ~10 steps."""
                nc.gpsimd.memset(ynp[bg][:, :, 0:MAXDISP], 0.0)
                nc.gpsimd.memset(ynp[bg][:, :, MAXDISP + W:], 0.0)
                raws = []
                sqs = []
                rss = []
                for is_y in (0, 1):
                    nm = "y" if is_y else "x"
                    raws.append(stagep.tile(
                        [128, HT, W], f32 if (bg == 0 and not is_y) else f16,
                        tag="raw", name=f"raw{nm}"))
                    sqs.append(stagep.tile(
                        [128, HT, W], f16, tag="sq", name=f"sq{nm}"))
                    rss.append(normp.tile(
                        [128, HT, W], f16, tag="rs", name=f"rs{nm}"))
                yield
                # pipelined per chunk: load -> square -> per-t ssum/rsqrt/mul
                for c0 in range(0, HT, _CHUNK):
                    sl = slice(c0, c0 + _CHUNK)
                    for is_y in (0, 1):
                        src_v = y_v if is_y else x_v
                        raw, sq, rs = raws[is_y], sqs[is_y], rss[is_y]
                        # f32 -> f16 cast happens inside the SWDGE DMA, so
                        # the norm multiply runs at DVE 2x on all-f16 operands.
                        # bg0's x rides SP uncast: Pool's serial desc-gen would
                        # otherwise pace the whole prologue.
                        if bg == 0 and not is_y:
                            nc.sync.dma_start(raw[:, sl], src_v[bg][:, sl])
                        else:
                            nc.gpsimd.dma_start(raw[:, sl], src_v[bg][:, sl])
                        nc.scalar.activation(
                            sq[:, sl].rearrange("p a b -> p (a b)"),
                            raw[:, sl].rearrange("p a b -> p (a b)"),
                            AF.Square,
                        )
                        for t in range(c0, c0 + _CHUNK):
                            ssum = npsp.tile([128, 512], f32, tag="ssum")
                            nc.tensor.matmul(
                                ssum[:, 0:W], s0_t[:], sq[:, t, :],
                                start=True, stop=True,
                            )
                            nc.scalar.activation(
                                rs[:, t, :], ssum[:, 0:W], AF.Abs_reciprocal_sqrt
                            )
                        if is_y:
                            nc.vector.tensor_mul(
                                ynp[bg][:, sl, MAXDISP:MAXDISP + W],
                                raw[:, sl], rs[:, sl],
                            )
                        else:
                            nc.vector.tensor_mul(
                                xn[bg][:, sl], raw[:, sl], rs[:, sl]
                            )
                        yield
                    if bg == 0 and c0 == 0:
                        emit_weight_consts()
                    if bg == 0:
                        head_min_chunk(c0, _CHUNK)
                # U = sum_c xn -> [96, W]; V = sum_c ynp -> [96, PW]
                ups = npsp.tile([128, 512], f32, tag="ssum", name="ups")
                for t in range(HT):
                    nc.tensor.matmul(
                        ups[0:H, 0:W], s6[t], xn[bg][:, t, :],
                        start=(t == 0), stop=(t == HT - 1),
                    )
                nc.scalar.activation(u1[bg][:], ups[0:H, 0:W], AF.Copy)
                yield
                vps = npsp.tile([128, 512], f32, tag="ssum", name="vps")
                for t in range(HT):
                    nc.tensor.matmul(
                        vps[0:H, 0:PW], s6[t], ynp[bg][:, t, :],
                        start=(t == 0), stop=(t == HT - 1),
                    )
                nc.scalar.activation(vp[bg][:], vps[0:H, 0:PW], AF.Copy)
                yield
                # A3[h, kk, j] = sum_c |xn_c[h, j]| for j < 46: the pad-strip
                # output (identical for every k in a triad), stored 3-wide.
                ax = normp.tile([128, HT, 48], f16, tag="ax")
                nc.scalar.activation(
                    ax[:, :, 0:47], xn[bg][:, :, 0:47], AF.Abs,
                )
                aps = npsp.tile([128, 512], f32, tag="ssum", name="aps")
                for t in range(HT):
                    nc.tensor.matmul(
                        aps[0:H, 0:47], s6[t], ax[:, t, 0:47],
                        start=(t == 0), stop=(t == HT - 1),
                    )
                nc.scalar.activation(
                    a3[bg][:],
                    aps[0:H, 0:47].unsqueeze(1).broadcast_to([H, 3, 47]),
                    AF.Copy,
                )
                yield

            # the first two triads' mins only need the xn/ynp rows of each
            # finished norm chunk, so they are emitted chunk-by-chunk inside
            # norm0 and fill DVE's otherwise idle prologue
            head_m3 = {}

            def head_min_chunk(c0, ch):
                for g in (0, 1):
                    if PATHS0[g] != "m":
                        continue
                    if g not in head_m3:
                        head_m3[g] = dpool.tile(
                            [128, HT, 3, W], f16, tag="d3", name="m3")
                    kmax = MAXDISP - 3 * g
                    base = MAXDISP - kmax
                    js = kmax - 2
                    w0 = W - js
                    nc.vector.tensor_tensor(
                        head_m3[g][:, c0:c0 + ch, 0:3, 0:w0],
                        xn[0][:, c0:c0 + ch, js:].unsqueeze(2)
                        .broadcast_to([128, ch, 3, w0]),
                        windows_desc(
                            ynp[0][:, c0:c0 + ch], base + js + 2, 3, w0),
                        ALU.min,
                    )

            for _ in emit_norm(0):
                pass
            norm1 = emit_norm(1)

            # ---------------- main loop: triads ----------------
            # 1-triad software-pipeline skew: triad g's PSUM->SBUF copy and
            # store are emitted AFTER triad g+1's elementwise/abs work, so the
            # in-order ACT queue runs the next abs before stalling on the
            # copy's PSUM dependency.
            pending = None

            def flush_pending():
                nonlocal pending
                if pending is None:
                    return
                p_bg, p_ps, p_ob, p_nwin, p_kmax, p_js, p_path = pending
                w0 = W - p_js
                if p_path == "t":
                    nc.vector.tensor_scalar(
                        p_ob[:, 0:p_nwin, p_js:], p_ps[:, 0:p_nwin, 0:w0],
                        0.0, None, op0=ALU.add,
                    )
                else:
                    nc.scalar.activation(
                        p_ob[:, 0:p_nwin, p_js:], p_ps[:, 0:p_nwin, 0:w0],
                        AF.Copy,
                    )
                klo = p_kmax - p_nwin + 1
                # stores ride the ACT hwdge queue: their data deps retire just
                # before them there, so the SEQ isn't parked on long semaphore
                # waits (a single SP queue serialized the whole kernel on
                # those waits).  The pad strip [0:js) goes straight from a3
                # via a second tiny DMA -- no engine op at all.
                nc.scalar.dma_start(
                    out_v[p_bg, klo:p_kmax + 1, :, p_js:].rearrange(
                        "k h w -> h k w"),
                    p_ob[:, 0:p_nwin, p_js:],
                )
                if p_js > 0:
                    nc.sync.dma_start(
                        out_v[p_bg, klo:p_kmax + 1, :, 0:p_js].rearrange(
                            "k h w -> h k w"),
                        a3[p_bg][:, 0:p_nwin, 0:p_js],
                    )
                pending = None

            for bg in range(BG_PER_CORE):
                for g in range(NG):
                    path = (PATHS0 if bg == 0 else PATHS1)[g]
                    if g < NG - 1:
                        kmax = MAXDISP - 3 * g       # triad k = kmax-2..kmax
                        nwin = 3
                        base = MAXDISP - kmax        # 48 - k for k = kmax
                    else:
                        kmax = 0
                        nwin = 1
                        base = MAXDISP
                    js = kmax - nwin + 1             # valid window: j >= js
                    w0 = W - js
                    # window kk corresponds to k = js + kk (ascending):
                    # ynp col = (j - js) + base + js + (nwin-1) - kk
                    ywin = windows_desc(
                        ynp[bg][:], base + js + (nwin - 1), nwin, w0)
                    xin_ap = (
                        xn[bg][:, :, js:].unsqueeze(2)
                        .broadcast_to([128, HT, nwin, w0])
                    )

                    ps = mmpsp.tile([H, 3, 512], f32, tag="ps")
                    ob = opool.tile([H, 3, W], f16, tag="ob")
                    if path == "m":
                        tail = bg == BG_PER_CORE - 1 and g >= NG - 2
                        if bg == 0 and g in head_m3:
                            m3 = head_m3[g]
                        else:
                            m3 = dpool.tile(
                                [128, HT, 3, W], f16, tag="d3", name="m3")
                        if bg == 0 and g in head_m3:
                            pass
                        elif tail:
                            # drain the pipeline: chunked mins let PE start
                            # before the whole min lands
                            for c0 in range(0, HT, 2):
                                nc.vector.tensor_tensor(
                                    m3[:, c0:c0 + 2, 0:nwin, 0:w0],
                                    xn[bg][:, c0:c0 + 2, js:].unsqueeze(2)
                                    .broadcast_to([128, 2, nwin, w0]),
                                    windows_desc(
                                        ynp[bg][:, c0:c0 + 2],
                                        base + js + (nwin - 1), nwin, w0),
                                    ALU.min,
                                )
                        else:
                            nc.vector.tensor_tensor(
                                m3[:, :, 0:nwin, 0:w0], xin_ap, ywin, ALU.min
                            )
                        flush_pending()
                        # per k: U + V_k + sum_c min (multi-bank matmul
                        # outputs fail the real ISA's s3d3 check, so one
                        # bank per matmul)
                        for kk in range(nwin):
                            vs = base + js + (nwin - 1) - kk
                            nc.tensor.matmul(
                                ps[:, kk, 0:w0], id96[:], u1[bg][:, js:],
                                start=True, stop=False,
                            )
                            nc.tensor.matmul(
                                ps[:, kk, 0:w0], id96[:],
                                vp[bg][:, vs:vs + w0],
                                start=False, stop=False,
                            )
                        for t in range(HT):
                            for kk in range(nwin):
                                nc.tensor.matmul(
                                    ps[:, kk, 0:w0], s6n[t],
                                    m3[:, t, kk, 0:w0],
                                    start=False, stop=(t == HT - 1),
                                )
                    else:  # 'q'
                        d3 = dpool.tile([128, HT, 3, W], f16, tag="d3", name="d3")
                        nc.vector.tensor_tensor(
                            d3[:, :, 0:nwin, 0:w0], xin_ap, ywin, ALU.subtract
                        )
                        q3 = qpool.tile([128, HT, 3, W], f8e4, tag="q3", name="q3")
                        # abs in per-2ht chunks so the PSUM-freeing copy of the
                        # previous group isn't head-of-line blocked behind one
                        # long ACT op
                        qch = HT // _QCH
                        if _QF:
                            flush_pending()
                        nc.scalar.activation(
                            q3[:, 0:qch, 0:nwin, 0:w0],
                            d3[:, 0:qch, 0:nwin, 0:w0],
                            AF.Abs,
                        )
                        if not _QF:
                            flush_pending()
                        for ci in range(1, _QCH):
                            nc.scalar.activation(
                                q3[:, ci * qch:(ci + 1) * qch, 0:nwin, 0:w0],
                                d3[:, ci * qch:(ci + 1) * qch, 0:nwin, 0:w0],
                                AF.Abs,
                            )
                        for kk in range(nwin):
                            for tp in range(HT // 2):
                                nc.tensor.matmul(
                                    ps[:, kk, 0:w0], s6dr[tp],
                                    q3[:, 2 * tp:2 * tp + 2, kk, 0:w0],
                                    start=(tp == 0), stop=(tp == HT // 2 - 1),
                                    perf_mode=PM.DoubleRow,
                                )

                    pending = (bg, ps, ob, nwin, kmax, js, path)
                    if bg == 0:
                        for _ in range(_PUMP):
                            next(norm1, None)
                if bg == 0:
                    for _ in norm1:
                        pass
            flush_pending()

    nc.compile()
    _PROG = nc
    return nc


def run(x, y, trace=False, trace_kwargs=None):
    """x, y: full [2, 64, 96, 320] f32. Returns (out [2,8,49,96,320] f32, res)."""
    from concourse import bass_utils

    nc = _build()
    consts = _constants()

    def _prep(a):
        a = np.asarray(a, np.float32).reshape(BG_TOTAL, CPG, HT, HH, W)
        return np.ascontiguousarray(a.transpose(0, 1, 3, 2, 4)).reshape(
            BG_TOTAL, 128, HT, W
        )

    xr = _prep(x)
    yr = _prep(y)
    in_maps = []
    for i in range(NCORES):
        sl = slice(i * BG_PER_CORE, (i + 1) * BG_PER_CORE)
        in_maps.append(
            {
                "x": np.ascontiguousarray(xr[sl]),
                "y": np.ascontiguousarray(yr[sl]),
                **consts,
            }
        )
    res = bass_utils.run_bass_kernel_spmd(
        nc,
        in_maps,
        core_ids=list(range(NCORES)),
        trace=trace,
        **(trace_kwargs or {}),
    )
    full = np.concatenate(
        [np.asarray(r["out"], np.float32) for r in res.results], axis=0
    )
    out = full.reshape(B, GROUP, D, H, W)
    return out, res


def kernel(x, y):
    out, _ = run(x, y, trace=False)
    return out
